# revision 22
# baseline (speedup 1.0000x reference)
"""NT-Xent (SimCLR) loss on 8 Trainium2 NeuronCores.

Math (validated against the reference formulation in f64):
  z = concat(z_i, z_j)                      [N=4096, D=512]
  zn = z / max(||z||, eps);  sim = zn@zn.T / T   (T=0.5, sim in [-2, 2])
  logits row i = sim row i minus the self-diagonal entry, so
    lse_i  = log(sum_{j!=i} exp(sim_ij))          (sim bounded => no shift)
    loss   = mean_i (lse_i - sim[i, partner(i)])
    rank_i = #{j != i : sim_ij > sim[i, partner(i)]}  (partner ties at 0)
    avg_rank = mean_i rank_i

Sharding: core r owns 512 rows of sim. Host normalizes z, quantizes
zn*S8 to fp8e4m3, transposes to [D, N] and permutes columns per core to
[partner-block | own-block | rest] so the partner / self diagonals sit
at fixed compile-time offsets (one NEFF for all cores; row stats are
column-permutation invariant). The gram G = (S8 zn)(S8 zn)^T carries
sim = k*G with k = (1/T)/S8^2 folded into the ScalarE activation scale.

PE runs fp8 DoubleRow matmuls (2 fp8 weights per cell -> 2x bf16
throughput): operands laid out [p, i, n], contraction K = s*256 +
i*128 + p over two super-tiles s. Inputs are staged as one SBUF tile
per (s, 1024-col quarter) so the first matmuls only depend on the q=0
DMAs. Each 2048-col PSUM quad (4 banks) gets one ScalarE exp with
fused row-sum (accum_out) and one VectorE greater-than-count with
fused row-sum, compared directly in G units against the extracted
partner diagonal (exact ties). Diagonals (pos, self) come from an
identity-mask multiply + reduce on the first quad. Device emits
per-row (S_full, d_G, pos_G, count); the host applies the exact
self-exclusion corrections and the final log/mean in f64.
"""

import numpy as np
import ml_dtypes

import concourse.bacc as bacc
import concourse.mybir as mybir
import concourse.tile as tile
from concourse.bass_utils import run_bass_kernel_spmd
from concourse.masks import make_identity

B = 2048
D = 512
N = 2 * B
NCORES = 8
RPC = N // NCORES  # rows of sim per core = 512
ST = D // 256      # DoubleRow super-tiles = 2 (K = 256 each)
MT = RPC // 128    # m tiles per core = 4
QUAD = 2048        # columns per PSUM quad-tile (4 banks of f32)
NQ = N // QUAD     # quads per m-tile = 2
S8 = 16.0          # fp8 pre-scale on zn
KSCALE = (1.0 / 0.5) / (S8 * S8)  # sim = KSCALE * G   (T = 0.5)

_f32 = mybir.dt.float32
_bf16 = mybir.dt.bfloat16
_fp8 = mybir.dt.float8e4

_NC_CACHE = {}


def _emit(tc):
    nc = tc.nc
    rhs_d = nc.dram_tensor("rhs", [ST, 128, 2, N], _fp8, kind="ExternalInput")[:]
    out_d = nc.dram_tensor("out", [128, 4 * MT + 1], _f32, kind="ExternalOutput")[:]

    with (
        tc.tile_pool(name="singles", bufs=1) as singles,
        tc.tile_pool(name="psum", bufs=2, space="PSUM") as psum,
        tc.tile_pool(name="scratch", bufs=3) as scratch,
        tc.tile_pool(name="acc", bufs=4) as acc,
    ):
        ident = singles.tile([128, 128], _f32)
        make_identity(nc, ident)
        kscale = singles.tile([128, 1], _f32)
        nc.vector.memset(kscale, KSCALE)

        # One SBUF tile per (super-tile s, 1024-col quarter q) so matmul
        # dependencies are per-quarter: the first matmuls start after the
        # two q=0 DMAs instead of the whole 2 MiB load.
        rhs_sb = [
            [
                singles.tile([128, 2, 1024], _fp8, tag=f"rhs{s}q{q}", name=f"rhs{s}q{q}")
                for q in range(4)
            ]
            for s in range(ST)
        ]
        for q in range(4):
            for s in range(ST):
                nc.sync.dma_start(
                    out=rhs_sb[s][q][:],
                    in_=rhs_d[s, :, :, q * 1024 : (q + 1) * 1024],
                )

        # Spare column 16: initialized once (GpSimd), ignored by the host.
        outs = singles.tile([128, 4 * MT + 1], _f32)
        nc.gpsimd.memset(outs[:, 4 * MT : 4 * MT + 1], 0.0)

        for t in range(MT):
            pos = acc.tile([128, 1], _f32, tag="pos")
            dself = acc.tile([128, 1], _f32, tag="dself")
            eacc = acc.tile([128, NQ], _f32, tag="eacc")
            cacc = acc.tile([128, NQ], _f32, tag="cacc")
            # lhsT = own-block columns (permuted cols 512..1023): quarter 0
            # of each super-tile, offset 512 + 128*t.
            lo = RPC + 128 * t
            for g in range(NQ):
                quad = psum.tile([128, QUAD], _f32, tag="ps", name="ps")
                for s in range(ST):
                    lhsT = rhs_sb[s][0][:, :, lo : lo + 128]
                    for h in range(4):
                        c = 4 * g + h  # 512-col chunk index
                        nc.tensor.matmul(
                            quad[:, 512 * h : 512 * (h + 1)],
                            lhsT,
                            rhs_sb[s][c // 2][:, :, (c % 2) * 512 : (c % 2) * 512 + 512],
                            start=(s == 0),
                            stop=(s == ST - 1),
                            perf_mode=mybir.MatmulPerfMode.DoubleRow,
                        )
                if g == 0:
                    # quad 0 holds both diagonals: partner block at cols
                    # 0..511, own block at cols 512..1023. Identity-mask
                    # extraction is exact (one nonzero per row).
                    for off, dst in ((128 * t, pos), (512 + 128 * t, dself)):
                        dj = scratch.tile([128, 128], _f32, tag="diagjunk", bufs=4)
                        nc.vector.tensor_mul(dj[:], quad[:, off : off + 128], ident[:])
                        nc.vector.reduce_sum(
                            out=dst[:], in_=dj[:], axis=mybir.AxisListType.X
                        )
                ej = scratch.tile([128, QUAD], _bf16, tag="ej")
                nc.scalar.activation(
                    out=ej[:],
                    in_=quad[:],
                    func=mybir.ActivationFunctionType.Exp,
                    scale=kscale[:],
                    accum_out=eacc[:, g : g + 1],
                )
                # count in G units against the extracted pos: the partner
                # element is bit-identical to pos -> strict > excludes it.
                cj = scratch.tile([128, QUAD], _bf16, tag="cj")
                nc.vector.tensor_scalar(
                    out=cj[:],
                    in0=quad[:],
                    scalar1=pos[:],
                    scalar2=None,
                    op0=mybir.AluOpType.is_gt,
                    op1=mybir.AluOpType.add,
                    accum_out=cacc[:, g : g + 1],
                )
            nc.vector.reduce_sum(
                out=outs[:, 4 * t : 4 * t + 1], in_=eacc[:], axis=mybir.AxisListType.X
            )
            nc.gpsimd.tensor_copy(out=outs[:, 4 * t + 1 : 4 * t + 2], in_=dself[:])
            nc.gpsimd.tensor_copy(out=outs[:, 4 * t + 2 : 4 * t + 3], in_=pos[:])
            nc.vector.reduce_sum(
                out=outs[:, 4 * t + 3 : 4 * t + 4],
                in_=cacc[:],
                axis=mybir.AxisListType.X,
            )

        nc.sync.dma_start(out=out_d, in_=outs[:])


def _build_nc():
    if "nc" in _NC_CACHE:
        return _NC_CACHE["nc"]
    # Bacc (not raw Bass): its compile() runs generate_event_semaphores,
    # which splits multi-sem waits into EventSemaphore instructions — the
    # hardware allows at most one sync wait per compute instruction.
    nc = bacc.Bacc("TRN2")
    with tile.TileContext(nc) as tc:
        _emit(tc)
    nc.compile()
    _NC_CACHE["nc"] = nc
    return nc


LAST_RESULT = None


def kernel(z_i, z_j, temperature=0.5):
    global LAST_RESULT
    z_i = np.asarray(z_i, dtype=np.float32)
    z_j = np.asarray(z_j, dtype=np.float32)
    assert z_i.shape == (B, D) and z_j.shape == (B, D)

    z = np.concatenate([z_i, z_j], axis=0)
    nrm = np.sqrt((z.astype(np.float64) ** 2).sum(axis=1, keepdims=True))
    nrm = np.maximum(nrm, 1e-8)
    zn = z / nrm
    zq = (zn * S8).astype(ml_dtypes.float8_e4m3)
    znT = np.ascontiguousarray(zq.T)  # [D, N]

    # device computes exp(kscale * G); host converts G-unit outputs with k
    k = (1.0 / float(temperature)) / (S8 * S8)

    rows = np.arange(N)
    in_maps = []
    for r in range(NCORES):
        own = rows[r * RPC : (r + 1) * RPC]
        part = (own + B) % N
        rest_mask = np.ones(N, dtype=bool)
        rest_mask[own] = False
        rest_mask[part] = False
        perm = np.concatenate([part, own, rows[rest_mask]])
        zp = znT[:, perm]  # [512, 4096]
        # DoubleRow layout: k = s*256 + i*128 + p  ->  [s, p, i, n]
        rhs = np.ascontiguousarray(zp.reshape(ST, 2, 128, N).transpose(0, 2, 1, 3))
        in_maps.append({"rhs": rhs})

    nc = _build_nc()
    res = run_bass_kernel_spmd(nc, in_maps, core_ids=list(range(NCORES)))
    LAST_RESULT = res

    tot_loss = 0.0
    tot_rank = 0.0
    for r in range(NCORES):
        o = np.asarray(res.results[r]["out"], dtype=np.float64)  # [128, 17]
        for t in range(MT):
            S = o[:, 4 * t + 0]
            dG = o[:, 4 * t + 1]
            pG = o[:, 4 * t + 2]
            cnt = o[:, 4 * t + 3]
            d = dG * k
            p = pG * k
            Sc = S - np.exp(d)  # exclude the self term
            tot_loss += (np.log(Sc) - p).sum()
            tot_rank += (cnt - (dG > pG)).sum()

    loss = np.array(tot_loss / N, dtype=np.float32)
    avg_rank = np.array(tot_rank / N, dtype=np.float32)
    return loss, avg_rank


# revision 23
# speedup vs baseline: 1.1841x; 1.1841x over previous
"""NT-Xent (SimCLR) loss on 8 Trainium2 NeuronCores.

Math (validated against the reference formulation in f64):
  z = concat(z_i, z_j)                      [N=4096, D=512]
  zn = z / max(||z||, eps);  sim = zn@zn.T / T   (T=0.5, sim in [-2, 2])
  logits row i = sim row i minus the self-diagonal entry, so
    lse_i  = log(sum_{j!=i} exp(sim_ij - C)) + C      with fixed C (sim bounded)
    loss   = mean_i (lse_i - sim[i, partner(i)])
    rank_i = #{j != i : sim_ij > sim[i, partner(i)]}  (partner ties at 0)
    avg_rank = mean_i rank_i

Sharding: core r owns 512 rows of sim. Host pre-normalizes z, scales by
sqrt(1/T) (so the bf16 gram IS sim), transposes to [D, N] and permutes
columns per core to [partner-block | own-block | rest] so the partner /
self diagonals sit at fixed compile-time offsets (one NEFF for all
cores; row stats are column-permutation invariant). Each core matmuls
its row block [512, 4096] in 512-col PSUM chunks, fuses exp+row-sum on
ScalarE (accum_out) and greater-than+count on VectorE (accum_out), and
extracts the two diagonals with an identity-mask multiply-reduce.
Device emits per-row (S_full, self_diag, pos, count); the host applies
the exact self-exclusion corrections and the final log/mean in f64.
"""

import numpy as np
import ml_dtypes

import concourse.bacc as bacc
import concourse.mybir as mybir
import concourse.tile as tile
from concourse.bass_utils import run_bass_kernel_spmd
from concourse.masks import make_identity

B = 2048
D = 512
N = 2 * B
NCORES = 8
RPC = N // NCORES  # rows of sim per core = 512
KT = D // 128      # k tiles = 4
MT = RPC // 128    # m tiles per core = 4
CHUNK = 512        # columns per PSUM chunk (one bank of f32)
NCH = N // CHUNK   # n chunks = 8
# Logsumexp shift: sim is bounded in [-2, 2] (cos/T, T=0.5), so exp(sim)
# never overflows f32 and no shift is needed (C = 0 keeps the device op
# bias-free, avoiding an extra cross-engine wait on the ACT instruction).
SHIFT = 0.0

_f32 = mybir.dt.float32
_bf16 = mybir.dt.bfloat16

_NC_CACHE = {}


def _emit(tc):
    nc = tc.nc
    rhs_d = nc.dram_tensor("rhs", [KT, 128, N], _bf16, kind="ExternalInput")[:]
    out_d = nc.dram_tensor("out", [128, 4 * MT + 1], _f32, kind="ExternalOutput")[:]

    with (
        tc.tile_pool(name="singles", bufs=1) as singles,
        tc.tile_pool(name="psum", bufs=8, space="PSUM") as psum,
        tc.tile_pool(name="scratch", bufs=3) as scratch,
        tc.tile_pool(name="acc", bufs=4) as acc,
    ):
        ident = singles.tile([128, 128], _f32)
        make_identity(nc, ident)

        # Stage the full [D, N] bf16 operand in SBUF: 4 k-tiles of
        # [128, 4096] (8 KiB/partition each). Split DMAs column-wise so
        # the first matmuls can start before the tail columns land.
        rhs_sb = []
        for k in range(KT):
            t = singles.tile([128, N], _bf16, tag=f"rhs{k}")
            for q in range(4):
                nc.sync.dma_start(
                    out=t[:, q * 1024 : (q + 1) * 1024],
                    in_=rhs_d[k, :, q * 1024 : (q + 1) * 1024],
                )
            rhs_sb.append(t)

        # One spare column (16): written by the sync-absorber op below and
        # ignored by the host. TensorTensor ISA encodes only ONE sync wait,
        # so the diag-extract muls must depend solely on the PE matmul; this
        # live TS op makes VectorE observe the GpSimd-built identity first.
        outs = singles.tile([128, 4 * MT + 1], _f32)
        nc.vector.tensor_scalar_mul(outs[:, 4 * MT : 4 * MT + 1], ident[:, 0:1], 0.0)

        for t in range(MT):
            pos = acc.tile([128, 1], _f32, tag="pos")
            dself = acc.tile([128, 1], _f32, tag="dself")
            eacc = acc.tile([128, NCH], _f32, tag="eacc")
            cacc = acc.tile([128, NCH], _f32, tag="cacc")
            # lhsT = own-block columns (permuted cols 512..1023) of this
            # m-tile; the same SBUF tiles feed both matmul operands.
            lo = RPC + 128 * t
            chunk_ps = {}
            for g in range(2):  # chunk groups of 4: fewer PE weight reloads
                group = range(4 * g, 4 * g + 4)
                for c in group:
                    chunk_ps[c] = psum.tile([128, CHUNK], _f32, tag="ps", name="ps")
                for k in range(KT):
                    lhsT = rhs_sb[k][:, lo : lo + 128]
                    for c in group:
                        nc.tensor.matmul(
                            chunk_ps[c][:],
                            lhsT,
                            rhs_sb[k][:, CHUNK * c : CHUNK * (c + 1)],
                            start=(k == 0),
                            stop=(k == KT - 1),
                        )
                for c in group:
                    ps = chunk_ps[c]
                    if c in (0, 1):
                        # c==0: partner diagonal -> pos; c==1: self
                        # diagonal -> dself. Exact: identity mask leaves
                        # one nonzero per row, sum of zeros is exact.
                        dj = scratch.tile([128, 128], _f32, tag="diagjunk", bufs=8)
                        nc.vector.tensor_mul(
                            dj[:], ps[:, 128 * t : 128 * (t + 1)], ident[:]
                        )
                        nc.vector.reduce_sum(
                            out=(pos if c == 0 else dself)[:],
                            in_=dj[:],
                            axis=mybir.AxisListType.X,
                        )
                    ej = scratch.tile([128, CHUNK], _bf16, tag="ej")
                    nc.scalar.activation(
                        out=ej[:],
                        in_=ps[:],
                        func=mybir.ActivationFunctionType.Exp,
                        accum_out=eacc[:, c : c + 1],
                    )
                    cj = scratch.tile([128, CHUNK], _bf16, tag="cj")
                    nc.vector.tensor_scalar(
                        out=cj[:],
                        in0=ps[:],
                        scalar1=pos[:],
                        scalar2=None,
                        op0=mybir.AluOpType.is_gt,
                        op1=mybir.AluOpType.add,
                        accum_out=cacc[:, c : c + 1],
                    )
            nc.vector.reduce_sum(
                out=outs[:, 4 * t : 4 * t + 1], in_=eacc[:], axis=mybir.AxisListType.X
            )
            nc.vector.tensor_copy(out=outs[:, 4 * t + 1 : 4 * t + 2], in_=dself[:])
            nc.vector.tensor_copy(out=outs[:, 4 * t + 2 : 4 * t + 3], in_=pos[:])
            nc.vector.reduce_sum(
                out=outs[:, 4 * t + 3 : 4 * t + 4], in_=cacc[:], axis=mybir.AxisListType.X
            )

        nc.sync.dma_start(out=out_d, in_=outs[:])


def _build_nc():
    if "nc" in _NC_CACHE:
        return _NC_CACHE["nc"]
    # Bacc (not raw Bass): its compile() runs generate_event_semaphores,
    # which splits multi-sem waits into EventSemaphore instructions — the
    # hardware allows at most one sync wait per compute instruction.
    nc = bacc.Bacc("TRN2")
    with tile.TileContext(nc) as tc:
        _emit(tc)
    nc.compile()
    _NC_CACHE["nc"] = nc
    return nc


LAST_RESULT = None


def kernel(z_i, z_j, temperature=0.5):
    global LAST_RESULT
    z_i = np.asarray(z_i, dtype=np.float32)
    z_j = np.asarray(z_j, dtype=np.float32)
    assert z_i.shape == (B, D) and z_j.shape == (B, D)

    z = np.concatenate([z_i, z_j], axis=0)
    nrm = np.sqrt((z.astype(np.float64) ** 2).sum(axis=1, keepdims=True))
    nrm = np.maximum(nrm, 1e-8)
    zn = z / nrm
    # scale by sqrt(1/T) so the gram matrix equals sim = cos/T directly
    znb = (zn * np.sqrt(1.0 / float(temperature))).astype(ml_dtypes.bfloat16)
    znT = np.ascontiguousarray(znb.T)  # [D, N]

    rows = np.arange(N)
    in_maps = []
    for r in range(NCORES):
        own = rows[r * RPC : (r + 1) * RPC]
        part = (own + B) % N
        rest_mask = np.ones(N, dtype=bool)
        rest_mask[own] = False
        rest_mask[part] = False
        perm = np.concatenate([part, own, rows[rest_mask]])
        rhs = np.ascontiguousarray(znT[:, perm]).reshape(KT, 128, N)
        in_maps.append({"rhs": rhs})

    nc = _build_nc()
    res = run_bass_kernel_spmd(nc, in_maps, core_ids=list(range(NCORES)))
    LAST_RESULT = res

    tot_loss = 0.0
    tot_rank = 0.0
    for r in range(NCORES):
        o = np.asarray(res.results[r]["out"], dtype=np.float64)  # [128, 17]; col 16 unused
        for t in range(MT):
            S = o[:, 4 * t + 0]
            d = o[:, 4 * t + 1]
            p = o[:, 4 * t + 2]
            cnt = o[:, 4 * t + 3]
            Sc = S - np.exp(d - SHIFT)  # exclude the self term
            tot_loss += (np.log(Sc) + SHIFT - p).sum()
            tot_rank += (cnt - (d > p)).sum()

    loss = np.array(tot_loss / N, dtype=np.float32)
    avg_rank = np.array(tot_rank / N, dtype=np.float32)
    return loss, avg_rank


# revision 25
# speedup vs baseline: 1.2199x; 1.0302x over previous
"""NT-Xent (SimCLR) loss on 8 Trainium2 NeuronCores.

Math (validated against the reference formulation in f64):
  z = concat(z_i, z_j)                      [N=4096, D=512]
  zn = z / max(||z||, eps);  sim = zn@zn.T / T   (T=0.5, sim in [-2, 2])
  logits row i = sim row i minus the self-diagonal entry, so
    lse_i  = log(sum_{j!=i} exp(sim_ij - C)) + C      with fixed C (sim bounded)
    loss   = mean_i (lse_i - sim[i, partner(i)])
    rank_i = #{j != i : sim_ij > sim[i, partner(i)]}  (partner ties at 0)
    avg_rank = mean_i rank_i

Sharding: core r owns 512 rows of sim. Host pre-normalizes z, scales by
sqrt(1/T) (so the bf16 gram IS sim), transposes to [D, N] and permutes
columns per core to [partner-block | own-block | rest] so the partner /
self diagonals sit at fixed compile-time offsets (one NEFF for all
cores; row stats are column-permutation invariant). Each core matmuls
its row block [512, 4096] in 512-col PSUM chunks, fuses exp+row-sum on
ScalarE (accum_out) and greater-than+count on VectorE (accum_out), and
extracts the two diagonals with an identity-mask multiply-reduce.
Device emits per-row (S_full, self_diag, pos, count); the host applies
the exact self-exclusion corrections and the final log/mean in f64.
"""

import numpy as np
import ml_dtypes

import concourse.bacc as bacc
import concourse.mybir as mybir
import concourse.tile as tile
from concourse.bass_utils import run_bass_kernel_spmd
from concourse.masks import make_identity

B = 2048
D = 512
N = 2 * B
NCORES = 8
RPC = N // NCORES  # rows of sim per core = 512
KT = D // 128      # k tiles = 4
MT = RPC // 128    # m tiles per core = 4
CHUNK = 512        # columns per PSUM chunk (one bank of f32)
NCH = N // CHUNK   # n chunks = 8
# Logsumexp shift: sim is bounded in [-2, 2] (cos/T, T=0.5), so exp(sim)
# never overflows f32 and no shift is needed (C = 0 keeps the device op
# bias-free, avoiding an extra cross-engine wait on the ACT instruction).
SHIFT = 0.0

_f32 = mybir.dt.float32
_bf16 = mybir.dt.bfloat16

_NC_CACHE = {}


def _emit(tc):
    nc = tc.nc
    rhs_d = nc.dram_tensor("rhs", [KT, 128, N], _bf16, kind="ExternalInput")[:]
    out_d = nc.dram_tensor("out", [128, 4 * MT + 1], _f32, kind="ExternalOutput")[:]

    with (
        tc.tile_pool(name="singles", bufs=1) as singles,
        tc.tile_pool(name="psum", bufs=8, space="PSUM") as psum,
        tc.tile_pool(name="scratch", bufs=3) as scratch,
        tc.tile_pool(name="acc", bufs=4) as acc,
    ):
        ident = singles.tile([128, 128], _f32)
        make_identity(nc, ident)

        # Stage the full [D, N] bf16 operand in SBUF: 4 k-tiles of
        # [128, 4096] (8 KiB/partition each). DMA issue on the SP queue is
        # ~0.7us per transfer, and the first matmul needs column-chunk q=0
        # of ALL FOUR k-tiles — so issue q-major (k inner): the 4 transfers
        # the first matmul depends on go out first, cutting ~7us of PE
        # head-idle vs k-major order.
        rhs_sb = [
            singles.tile([128, N], _bf16, tag=f"rhs{k}", name=f"rhs{k}")
            for k in range(KT)
        ]
        for q in range(4):
            for k in range(KT):
                nc.sync.dma_start(
                    out=rhs_sb[k][:, q * 1024 : (q + 1) * 1024],
                    in_=rhs_d[k, :, q * 1024 : (q + 1) * 1024],
                )

        # One spare column (16): written by the sync-absorber op below and
        # ignored by the host. TensorTensor ISA encodes only ONE sync wait,
        # so the diag-extract muls must depend solely on the PE matmul; this
        # live TS op makes VectorE observe the GpSimd-built identity first.
        outs = singles.tile([128, 4 * MT + 1], _f32)
        nc.vector.tensor_scalar_mul(outs[:, 4 * MT : 4 * MT + 1], ident[:, 0:1], 0.0)

        for t in range(MT):
            pos = acc.tile([128, 1], _f32, tag="pos")
            dself = acc.tile([128, 1], _f32, tag="dself")
            eacc = acc.tile([128, NCH], _f32, tag="eacc")
            cacc = acc.tile([128, NCH], _f32, tag="cacc")
            # lhsT = own-block columns (permuted cols 512..1023) of this
            # m-tile; the same SBUF tiles feed both matmul operands.
            lo = RPC + 128 * t
            chunk_ps = {}
            for g in range(2):  # chunk groups of 4: fewer PE weight reloads
                group = range(4 * g, 4 * g + 4)
                for c in group:
                    chunk_ps[c] = psum.tile([128, CHUNK], _f32, tag="ps", name="ps")
                for k in range(KT):
                    lhsT = rhs_sb[k][:, lo : lo + 128]
                    for c in group:
                        nc.tensor.matmul(
                            chunk_ps[c][:],
                            lhsT,
                            rhs_sb[k][:, CHUNK * c : CHUNK * (c + 1)],
                            start=(k == 0),
                            stop=(k == KT - 1),
                        )
                for c in group:
                    ps = chunk_ps[c]
                    if c in (0, 1):
                        # c==0: partner diagonal -> pos; c==1: self
                        # diagonal -> dself. Exact: identity mask leaves
                        # one nonzero per row, sum of zeros is exact.
                        dj = scratch.tile([128, 128], _f32, tag="diagjunk", bufs=8)
                        nc.vector.tensor_mul(
                            dj[:], ps[:, 128 * t : 128 * (t + 1)], ident[:]
                        )
                        nc.vector.reduce_sum(
                            out=(pos if c == 0 else dself)[:],
                            in_=dj[:],
                            axis=mybir.AxisListType.X,
                        )
                    ej = scratch.tile([128, CHUNK], _bf16, tag="ej")
                    nc.scalar.activation(
                        out=ej[:],
                        in_=ps[:],
                        func=mybir.ActivationFunctionType.Exp,
                        accum_out=eacc[:, c : c + 1],
                    )
                    cj = scratch.tile([128, CHUNK], _bf16, tag="cj")
                    nc.vector.tensor_scalar(
                        out=cj[:],
                        in0=ps[:],
                        scalar1=pos[:],
                        scalar2=None,
                        op0=mybir.AluOpType.is_gt,
                        op1=mybir.AluOpType.add,
                        accum_out=cacc[:, c : c + 1],
                    )
            nc.vector.reduce_sum(
                out=outs[:, 4 * t : 4 * t + 1], in_=eacc[:], axis=mybir.AxisListType.X
            )
            # tiny [128,1] copies go to the otherwise-idle GpSimd queue so
            # they don't serialize behind VectorE's count pass
            nc.gpsimd.tensor_copy(out=outs[:, 4 * t + 1 : 4 * t + 2], in_=dself[:])
            nc.gpsimd.tensor_copy(out=outs[:, 4 * t + 2 : 4 * t + 3], in_=pos[:])
            nc.vector.reduce_sum(
                out=outs[:, 4 * t + 3 : 4 * t + 4], in_=cacc[:], axis=mybir.AxisListType.X
            )

        nc.sync.dma_start(out=out_d, in_=outs[:])


def _build_nc():
    if "nc" in _NC_CACHE:
        return _NC_CACHE["nc"]
    # Bacc (not raw Bass): its compile() runs generate_event_semaphores,
    # which splits multi-sem waits into EventSemaphore instructions — the
    # hardware allows at most one sync wait per compute instruction.
    nc = bacc.Bacc("TRN2")
    with tile.TileContext(nc) as tc:
        _emit(tc)
    nc.compile()
    _NC_CACHE["nc"] = nc
    return nc


LAST_RESULT = None


def kernel(z_i, z_j, temperature=0.5):
    global LAST_RESULT
    z_i = np.asarray(z_i, dtype=np.float32)
    z_j = np.asarray(z_j, dtype=np.float32)
    assert z_i.shape == (B, D) and z_j.shape == (B, D)

    z = np.concatenate([z_i, z_j], axis=0)
    nrm = np.sqrt((z.astype(np.float64) ** 2).sum(axis=1, keepdims=True))
    nrm = np.maximum(nrm, 1e-8)
    zn = z / nrm
    # scale by sqrt(1/T) so the gram matrix equals sim = cos/T directly
    znb = (zn * np.sqrt(1.0 / float(temperature))).astype(ml_dtypes.bfloat16)
    znT = np.ascontiguousarray(znb.T)  # [D, N]

    rows = np.arange(N)
    in_maps = []
    for r in range(NCORES):
        own = rows[r * RPC : (r + 1) * RPC]
        part = (own + B) % N
        rest_mask = np.ones(N, dtype=bool)
        rest_mask[own] = False
        rest_mask[part] = False
        perm = np.concatenate([part, own, rows[rest_mask]])
        rhs = np.ascontiguousarray(znT[:, perm]).reshape(KT, 128, N)
        in_maps.append({"rhs": rhs})

    nc = _build_nc()
    res = run_bass_kernel_spmd(nc, in_maps, core_ids=list(range(NCORES)))
    LAST_RESULT = res

    tot_loss = 0.0
    tot_rank = 0.0
    for r in range(NCORES):
        o = np.asarray(res.results[r]["out"], dtype=np.float64)  # [128, 17]; col 16 unused
        for t in range(MT):
            S = o[:, 4 * t + 0]
            d = o[:, 4 * t + 1]
            p = o[:, 4 * t + 2]
            cnt = o[:, 4 * t + 3]
            Sc = S - np.exp(d - SHIFT)  # exclude the self term
            tot_loss += (np.log(Sc) + SHIFT - p).sum()
            tot_rank += (cnt - (d > p)).sum()

    loss = np.array(tot_loss / N, dtype=np.float32)
    avg_rank = np.array(tot_rank / N, dtype=np.float32)
    return loss, avg_rank
